# revision 16
# baseline (speedup 1.0000x reference)
"""Masked ternary linear layer on 8 TRN2 NeuronCores.

out = x @ ternarize((weight_base + weight_tag) * expand(tile_mask)).T + bias

Sharding: tensor-parallel column sharding along out_features. Each core
gets a 1024-wide slice of the weights, x is replicated; the 8 per-core
[128, 1024] outputs are concatenated on host.

The ternary weight matrix (values in {-1,0,+1}) is a pure function of
the inputs, computed once on the host exactly as the reference does and
shipped to the device packed 2 bits per weight. The 2-bit code of each
weight is chosen so that, shifted to the top bits of its byte, it IS
the fp8e4m3 bit pattern of 2*w:

   w=+1 -> 01 -> 0x40 = +2.0    w=-1 -> 11 -> 0xC0 = -2.0    w=0 -> 00

Four weights (4 k-chunks) pack per byte; uint16 lanes cover two
adjacent output columns. Device decode is ONE fused DVE op per
[128, 1024] weight plane, on uint16 lanes at half the element count:

   plane_q = (packed << 2q) & 0xC0C0        # uint16 in/out, bitVec ops

and the result is bitcast to fp8e4 for the matmul (moving operand fp8,
full PE rate). x is pre-scaled by 0.5 on the host so x/2 @ (2w) = x@w;
bias is seeded into PSUM with a K=1 ones matmul.

The first NDR chunks additionally run in fp8 DoubleRow mode (x cast to
fp8e4m3 for those chunks only, 2 chunks per matmul at 2x PE rate). The
fp8 quantization of x over NDR/KC of the contraction adds
~2.6% * sqrt(NDR/KC) relative error; NDR=16 keeps the total ~1.3e-2,
under the 2e-2 gate. DR chunks go first: they double the work done
while the PE clock is still ramping (full clock arrives ~8us into the
program regardless of activity).

Per-core budget: DMA 2.1 MB packed weights + 2.4 MB x + 0.25 MB out;
DVE 64 decode ops ~ 14 us; PE 48 normal + 8 DR matmul-chunks.
"""

import numpy as np

import concourse.bass as bass
import concourse.mybir as mybir
from concourse import bacc
from concourse.bass_utils import run_bass_kernel_spmd
from concourse.tile import TileContext

N_CORES = 8
BATCH = 128
IN_FEATURES = 8192
OUT_FEATURES = 8192
TILE = 64
THRESH = 0.3
OUT_CORE = OUT_FEATURES // N_CORES

_F32 = mybir.dt.float32
_BF16 = mybir.dt.bfloat16
_U16 = mybir.dt.uint16
_FP8 = mybir.dt.float8e4


def _ternarize(weight_base, weight_tag, tile_mask):
    """Exact reference ternarization (f32 ops), -> int8 in {-1,0,1}."""
    mask = np.repeat(np.repeat(tile_mask, TILE, 0), TILE, 1).astype(np.float32)
    w = np.clip((weight_base + weight_tag) * mask, -1.0, 1.0)
    w = np.where(np.abs(w) < THRESH, 0.0, np.sign(w))
    return w.astype(np.int8)


def build_graph(in_features: int, out_core: int, batch: int = BATCH,
                ndr: int = 16,            # chunks done in fp8 DoubleRow
                wd_bufs: int = 20, pk_bufs: int = 4) -> bacc.Bacc:
    KC = in_features // 128          # contraction chunks of 128 rows
    GC = KC // 4                     # packed groups (4 chunks per byte)
    LAN = out_core // 2              # uint16 lanes per packed plane
    assert ndr % 4 == 0              # DR region = whole packed groups

    nc = bacc.Bacc("TRN2", target_bir_lowering=False, debug=False,
                   num_devices=N_CORES)
    # xTc[p, k*batch + b] = x[b, k*128 + p] * 0.5   (bf16)
    xTc = nc.dram_tensor("xTc", [128, KC * batch], _BF16,
                         kind="ExternalInput").ap()
    wpk = nc.dram_tensor("wpk", [128, GC * LAN], _U16,
                         kind="ExternalInput").ap()
    x8c = None
    if ndr:
        x8c = nc.dram_tensor("x8c", [128, ndr * batch], _FP8,
                             kind="ExternalInput").ap()
    bias = nc.dram_tensor("bias", [1, out_core], _BF16,
                          kind="ExternalInput").ap()
    out = nc.dram_tensor("out", [batch, out_core], _BF16,
                         kind="ExternalOutput").ap()

    slices = [(o, min(512, out_core - o)) for o in range(0, out_core, 512)]
    xpieces = [4, 4, 8, 16, 32]
    assert sum(xpieces) == KC

    with TileContext(nc) as tc:
        with (
            tc.tile_pool(name="persist", bufs=1) as persist,
            tc.tile_pool(name="pk", bufs=pk_bufs) as pkp,
            tc.tile_pool(name="wd", bufs=wd_bufs) as wdp,
            tc.tile_pool(name="outp", bufs=1) as outp,
            tc.tile_pool(name="psum", bufs=1, space="PSUM") as psp,
        ):
            bias_sb = persist.tile([1, out_core], _BF16)
            xT_sb = persist.tile([128, KC, batch], _BF16)
            x8_sb = (persist.tile([128, ndr, batch], _FP8, name="x8_sb")
                     if ndr else None)
            pk_ts = []

            def dma_pk2(q):
                g = 2 * len(pk_ts)
                pk_t = pkp.tile([128, 2, LAN], _U16, name=f"pk{g}")
                q.dma_start(out=pk_t[:],
                            in_=wpk[:, g * LAN:(g + 2) * LAN].rearrange(
                                "p (g l) -> p g l", l=LAN))
                pk_ts.append(pk_t)

            def dma_x(p, q):
                a = sum(xpieces[:p])
                b = a + xpieces[p]
                q.dma_start(
                    out=xT_sb[:, a:b, :],
                    in_=xTc[:, a * batch:b * batch].rearrange(
                        "p (k b) -> p k b", b=batch))

            # Deadline-ordered DMA schedule on the two HWDGE rings
            # (v2 layout, which ran starvation-free).
            dma_pk2(nc.sync)               # A: groups 0-1
            if ndr:
                nc.scalar.dma_start(       # B: fp8 x for the DR chunks
                    out=x8_sb[:],
                    in_=x8c[:].rearrange("p (k b) -> p k b", b=batch))
            nc.sync.dma_start(out=bias_sb[:], in_=bias[:])   # A
            dma_pk2(nc.scalar)             # B: groups 2-3
            dma_x(0, nc.scalar)            # B
            dma_x(1, nc.scalar)            # B
            dma_x(2, nc.sync)              # A
            dma_pk2(nc.sync)               # A: groups 4-5
            dma_pk2(nc.scalar)             # B: groups 6-7
            dma_pk2(nc.sync)               # A: groups 8-9
            dma_pk2(nc.scalar)             # B: groups 10-11
            dma_x(4, nc.sync)              # A
            dma_x(3, nc.scalar)            # B
            dma_pk2(nc.sync)               # A: groups 12-13
            dma_pk2(nc.scalar)             # B: groups 14-15

            ones_row = persist.tile([1, 128], _BF16)
            nc.vector.memset(ones_row[:], 1.0)

            ps = [psp.tile([128, w], _F32, name=f"ps{i}")
                  for i, (_, w) in enumerate(slices)]
            # bias seeds the accumulators; runs while the PE waits for
            # the first weight/x transfers anyway
            for si, (o0, wd_) in enumerate(slices):
                nc.tensor.matmul(ps[si][:], ones_row[:],
                                 bias_sb[:, o0:o0 + wd_],
                                 start=True, stop=False)

            def decode(out_ap, pk_t, gg, qq):
                nc.vector.tensor_scalar(
                    out=out_ap, in0=pk_t[:, gg, :],
                    scalar1=2 * qq, scalar2=0xC0C0,
                    op0=mybir.AluOpType.logical_shift_left,
                    op1=mybir.AluOpType.bitwise_and)

            # --- DR region: pairs of chunks, both operands fp8 ---
            for c in range(ndr // 2):
                k0 = 2 * c
                wd2 = wdp.tile([128, 2, LAN], _U16, name="wd2")
                for i in range(2):
                    k = k0 + i
                    decode(wd2[:, i, :], pk_ts[k // 8], (k // 4) % 2, k % 4)
                rhs8 = wd2[:].bitcast(_FP8)          # [128, 2, 2*LAN]
                for si, (o0, w_) in enumerate(slices):
                    nc.tensor.matmul(ps[si][:], x8_sb[:, k0:k0 + 2, :],
                                     rhs8[:, :, o0:o0 + w_],
                                     start=False, stop=False,
                                     perf_mode=mybir.MatmulPerfMode.DoubleRow)
            # --- normal region: bf16 x, fp8 weights ---
            for k in range(ndr, KC):
                wd_t = wdp.tile([128, LAN], _U16)
                decode(wd_t[:], pk_ts[k // 8], (k // 4) % 2, k % 4)
                last = (k == KC - 1)
                for si, (o0, w_) in enumerate(slices):
                    rhs = wd_t[:, o0 // 2:(o0 + w_) // 2].bitcast(_FP8)
                    nc.tensor.matmul(ps[si][:], xT_sb[:, k, :],
                                     rhs, start=False, stop=last)

            # split evacuation: slice 0 on DVE, slice 1 on Act, each
            # followed immediately by its own half of the output DMA
            out_sb = outp.tile([128, out_core], _BF16)
            nc.vector.tensor_copy(out=out_sb[:, 0:512], in_=ps[0][:])
            nc.sync.dma_start(out=out[:, 0:512], in_=out_sb[:, 0:512])
            nc.scalar.copy(out=out_sb[:, 512:1024], in_=ps[1][:])
            nc.scalar.dma_start(out=out[:, 512:1024],
                                in_=out_sb[:, 512:1024])

    nc.compile()
    return nc


def shard_inputs(x, weight_base, weight_tag, tile_mask, bias,
                 mode="p2f8", ndr=16):
    """Host-side data prep: ternarize, shard, pack, re-layout."""
    import ml_dtypes
    in_features = x.shape[1]
    batch = x.shape[0]
    out_features = weight_base.shape[0]
    out_core = out_features // N_CORES
    KC = in_features // 128
    GC = KC // 4

    tern = _ternarize(np.asarray(weight_base, np.float32),
                      np.asarray(weight_tag, np.float32),
                      np.asarray(tile_mask, np.float32))

    xs = np.asarray(x, np.float32) * 0.5
    xT = xs.T.reshape(KC, 128, batch).transpose(1, 0, 2)   # [p, k, b]
    xTc = np.ascontiguousarray(
        xT.reshape(128, KC * batch).astype(ml_dtypes.bfloat16))
    x8c = np.ascontiguousarray(
        xT[:, :ndr, :].reshape(128, ndr * batch)
        .astype(ml_dtypes.float8_e4m3)) if ndr else None
    bias_bf = np.asarray(bias, np.float32).astype(ml_dtypes.bfloat16)

    in_maps = []
    for c in range(N_CORES):
        o0, o1 = c * out_core, (c + 1) * out_core
        tt = tern[o0:o1, :].T                       # [in, out_core] int8
        m = {"xTc": xTc,
             "bias": np.ascontiguousarray(bias_bf[o0:o1].reshape(1, -1))}
        if x8c is not None:
            m["x8c"] = x8c
        code = (tt & 3).astype(np.uint16)           # -1->3, 0->0, +1->1
        code = code.reshape(GC, 4, 128, out_core)
        byte = np.zeros((GC, 128, out_core), np.uint16)
        for qq in range(4):
            byte |= code[:, qq] << np.uint16(6 - 2 * qq)
        # uint16 lane jj = cols (2jj, 2jj+1), little-endian
        pk = byte[:, :, 0::2] | (byte[:, :, 1::2] << np.uint16(8))
        m["wpk"] = np.ascontiguousarray(
            pk.transpose(1, 0, 2).reshape(128, GC * (out_core // 2)))
        in_maps.append(m)
    return in_maps, mode


_GRAPH_CACHE = {}


def _get_graph(in_features, out_core, batch, **kw):
    key = (in_features, out_core, batch, tuple(sorted(kw.items())))
    if key not in _GRAPH_CACHE:
        _GRAPH_CACHE[key] = build_graph(in_features, out_core, batch, **kw)
    return _GRAPH_CACHE[key]


def run_sharded(in_maps, trace=False, mode="p2f8", **kw):
    batch = BATCH
    in_features = in_maps[0]["xTc"].shape[1] * 128 // batch
    out_core = in_maps[0]["wpk"].shape[1] * 8 // (in_features // 128)
    if "x8c" in in_maps[0]:
        kw.setdefault("ndr", in_maps[0]["x8c"].shape[1] // batch)
    else:
        kw.setdefault("ndr", 0)
    nc = _get_graph(in_features, out_core, batch, **kw)
    res = run_bass_kernel_spmd(nc, in_maps, core_ids=list(range(N_CORES)),
                               trace=trace)
    full = np.concatenate([res.results[i]["out"] for i in range(N_CORES)],
                          axis=1)
    return full, res


def kernel(x, weight_base, weight_tag, tile_mask, bias):
    x = np.ascontiguousarray(np.asarray(x, dtype=np.float32))
    weight_base = np.ascontiguousarray(np.asarray(weight_base, np.float32))
    weight_tag = np.ascontiguousarray(np.asarray(weight_tag, np.float32))
    tile_mask = np.ascontiguousarray(np.asarray(tile_mask, np.float32))
    bias = np.ascontiguousarray(np.asarray(bias, np.float32))
    in_maps, mode = shard_inputs(x, weight_base, weight_tag, tile_mask,
                                 bias)
    full, _ = run_sharded(in_maps, trace=False, mode=mode)
    return np.ascontiguousarray(full.astype(np.float32))


# revision 18
# speedup vs baseline: 1.1890x; 1.1890x over previous
"""Masked ternary linear layer on 8 TRN2 NeuronCores.

out = x @ ternarize((weight_base + weight_tag) * expand(tile_mask)).T + bias

Sharding: tensor-parallel column sharding along out_features. Each core
gets a 1024-wide slice of the weights, x is replicated; the 8 per-core
[128, 1024] outputs are concatenated on host.

The ternary weight matrix (values in {-1,0,+1}) is a pure function of
the inputs, computed once on the host exactly as the reference does and
shipped to the device packed 2 bits per weight. The 2-bit code of each
weight is chosen so that, shifted to the top bits of its byte, it IS
the fp8e4m3 bit pattern of 2*w:

   w=+1 -> 01 -> 0x40 = +2.0    w=-1 -> 11 -> 0xC0 = -2.0    w=0 -> 00

Four weights (4 k-chunks) pack per byte; uint16 lanes cover two
adjacent output columns. Device decode is ONE fused DVE op per
[128, 1024] weight plane, on uint16 lanes at half the element count:

   plane_q = (packed << 2q) & 0xC0C0        # uint16 in/out, bitVec ops

and the result is bitcast to fp8e4 for the matmul (moving operand fp8,
full PE rate). x is pre-scaled by 0.5 on the host so x/2 @ (2w) = x@w;
bias is seeded into PSUM with a K=1 ones matmul.

The first NDR chunks additionally run in fp8 DoubleRow mode (x cast to
fp8e4m3 for those chunks only, 2 chunks per matmul at 2x PE rate). The
fp8 quantization of x over NDR/KC of the contraction adds
~2.6% * sqrt(NDR/KC) relative error; NDR=16 keeps the total ~1.3e-2,
under the 2e-2 gate. DR chunks go first: they double the work done
while the PE clock is still ramping (full clock arrives ~8us into the
program regardless of activity).

Per-core budget: DMA 2.1 MB packed weights + 2.4 MB x + 0.25 MB out;
DVE 64 decode ops ~ 14 us; PE 48 normal + 8 DR matmul-chunks.
"""

import numpy as np

import concourse.bass as bass
import concourse.mybir as mybir
from concourse import bacc
from concourse.bass_utils import run_bass_kernel_spmd
from concourse.tile import TileContext

N_CORES = 8
BATCH = 128
IN_FEATURES = 8192
OUT_FEATURES = 8192
TILE = 64
THRESH = 0.3
OUT_CORE = OUT_FEATURES // N_CORES

_F32 = mybir.dt.float32
_BF16 = mybir.dt.bfloat16
_U16 = mybir.dt.uint16
_FP8 = mybir.dt.float8e4


def _ternarize(weight_base, weight_tag, tile_mask):
    """Exact reference ternarization (f32 ops), -> int8 in {-1,0,1}."""
    mask = np.repeat(np.repeat(tile_mask, TILE, 0), TILE, 1).astype(np.float32)
    w = np.clip((weight_base + weight_tag) * mask, -1.0, 1.0)
    w = np.where(np.abs(w) < THRESH, 0.0, np.sign(w))
    return w.astype(np.int8)


def build_graph(in_features: int, out_core: int, batch: int = BATCH,
                ndr: int = 16,            # chunks done in fp8 DoubleRow
                wd_bufs: int = 20, pk_bufs: int = 4) -> bacc.Bacc:
    KC = in_features // 128          # contraction chunks of 128 rows
    GC = KC // 4                     # packed groups (4 chunks per byte)
    LAN = out_core // 2              # uint16 lanes per packed plane
    assert ndr % 4 == 0              # DR region = whole packed groups

    nc = bacc.Bacc("TRN2", target_bir_lowering=False, debug=False,
                   num_devices=N_CORES)
    # xTc[p, k*batch + b] = x[b, k*128 + p] * 0.5   (bf16)
    xTc = nc.dram_tensor("xTc", [128, KC * batch], _BF16,
                         kind="ExternalInput").ap()
    wpk = nc.dram_tensor("wpk", [128, GC * LAN], _U16,
                         kind="ExternalInput").ap()
    x8c = None
    if ndr:
        x8c = nc.dram_tensor("x8c", [128, ndr * batch], _FP8,
                             kind="ExternalInput").ap()
    bias = nc.dram_tensor("bias", [1, out_core], _BF16,
                          kind="ExternalInput").ap()
    out = nc.dram_tensor("out", [batch, out_core], _BF16,
                         kind="ExternalOutput").ap()

    slices = [(o, min(512, out_core - o)) for o in range(0, out_core, 512)]
    # bf16 x pieces cover only the non-DR chunks [ndr, KC)
    xpieces = [(ndr, ndr + 8), (ndr + 8, ndr + 24), (ndr + 24, KC)] \
        if ndr else [(0, 4), (4, 8), (8, 16), (16, 32), (32, KC)]

    with TileContext(nc) as tc:
        with (
            tc.tile_pool(name="persist", bufs=1) as persist,
            tc.tile_pool(name="pk", bufs=pk_bufs) as pkp,
            tc.tile_pool(name="wd", bufs=wd_bufs) as wdp,
            tc.tile_pool(name="outp", bufs=1) as outp,
            tc.tile_pool(name="psum", bufs=1, space="PSUM") as psp,
        ):
            bias_sb = persist.tile([1, out_core], _BF16)
            xT_sb = persist.tile([128, KC, batch], _BF16)
            x8_sb = (persist.tile([128, ndr, batch], _FP8, name="x8_sb")
                     if ndr else None)
            pk_ts = []

            def dma_pk2(q):
                g = 2 * len(pk_ts)
                pk_t = pkp.tile([128, 2, LAN], _U16, name=f"pk{g}")
                q.dma_start(out=pk_t[:],
                            in_=wpk[:, g * LAN:(g + 2) * LAN].rearrange(
                                "p (g l) -> p g l", l=LAN))
                pk_ts.append(pk_t)

            def dma_x(p, q):
                a, b = xpieces[p]
                q.dma_start(
                    out=xT_sb[:, a:b, :],
                    in_=xTc[:, a * batch:b * batch].rearrange(
                        "p (k b) -> p k b", b=batch))

            # Deadline-ordered DMA schedule on the two HWDGE rings.
            if ndr:
                # A: bias pk01 pk23 xb0 pk45 pk89 pk1213
                # B: x8c pk67 xb1 pk1011 xb2 pk1415
                nc.sync.dma_start(out=bias_sb[:], in_=bias[:])
                nc.scalar.dma_start(
                    out=x8_sb[:],
                    in_=x8c[:].rearrange("p (k b) -> p k b", b=batch))
                dma_pk2(nc.sync)           # groups 0-1
                dma_pk2(nc.sync)           # groups 2-3
                dma_x(0, nc.sync)
                dma_pk2(nc.scalar)         # groups 4-5  (chunks 16-23)
                dma_pk2(nc.scalar)         # groups 6-7
                dma_x(1, nc.sync)
                dma_pk2(nc.sync)           # groups 8-9
                dma_pk2(nc.scalar)         # groups 10-11
                dma_x(2, nc.scalar)
                dma_pk2(nc.sync)           # groups 12-13
                dma_pk2(nc.scalar)         # groups 14-15
            else:
                dma_pk2(nc.sync)
                nc.sync.dma_start(out=bias_sb[:], in_=bias[:])
                dma_pk2(nc.scalar)
                dma_x(0, nc.scalar)
                dma_x(1, nc.scalar)
                dma_x(2, nc.sync)
                dma_pk2(nc.sync)
                dma_pk2(nc.scalar)
                dma_pk2(nc.sync)
                dma_pk2(nc.scalar)
                dma_x(4, nc.sync)
                dma_x(3, nc.scalar)
                dma_pk2(nc.sync)
                dma_pk2(nc.scalar)

            ones_row = persist.tile([1, 128], _BF16)
            nc.vector.memset(ones_row[:], 1.0)

            ps = [psp.tile([128, w], _F32, name=f"ps{i}")
                  for i, (_, w) in enumerate(slices)]
            # bias seeds the accumulators; runs while the PE waits for
            # the first weight/x transfers anyway
            for si, (o0, wd_) in enumerate(slices):
                nc.tensor.matmul(ps[si][:], ones_row[:],
                                 bias_sb[:, o0:o0 + wd_],
                                 start=True, stop=False)

            def decode(out_ap, pk_t, gg, qq):
                nc.vector.tensor_scalar(
                    out=out_ap, in0=pk_t[:, gg, :],
                    scalar1=2 * qq, scalar2=0xC0C0,
                    op0=mybir.AluOpType.logical_shift_left,
                    op1=mybir.AluOpType.bitwise_and)

            # --- DR region: pairs of chunks, both operands fp8 ---
            for c in range(ndr // 2):
                k0 = 2 * c
                wd2 = wdp.tile([128, 2, LAN], _U16, name="wd2")
                for i in range(2):
                    k = k0 + i
                    decode(wd2[:, i, :], pk_ts[k // 8], (k // 4) % 2, k % 4)
                rhs8 = wd2[:].bitcast(_FP8)          # [128, 2, 2*LAN]
                for si, (o0, w_) in enumerate(slices):
                    nc.tensor.matmul(ps[si][:], x8_sb[:, k0:k0 + 2, :],
                                     rhs8[:, :, o0:o0 + w_],
                                     start=False, stop=False,
                                     perf_mode=mybir.MatmulPerfMode.DoubleRow)
            # --- normal region: bf16 x, fp8 weights ---
            for k in range(ndr, KC):
                wd_t = wdp.tile([128, LAN], _U16)
                decode(wd_t[:], pk_ts[k // 8], (k // 4) % 2, k % 4)
                last = (k == KC - 1)
                for si, (o0, w_) in enumerate(slices):
                    rhs = wd_t[:, o0 // 2:(o0 + w_) // 2].bitcast(_FP8)
                    nc.tensor.matmul(ps[si][:], xT_sb[:, k, :],
                                     rhs, start=False, stop=last)

            # split evacuation: slice 0 on DVE, slice 1 on Act, each
            # followed immediately by its own half of the output DMA
            out_sb = outp.tile([128, out_core], _BF16)
            nc.vector.tensor_copy(out=out_sb[:, 0:512], in_=ps[0][:])
            nc.sync.dma_start(out=out[:, 0:512], in_=out_sb[:, 0:512])
            nc.scalar.copy(out=out_sb[:, 512:1024], in_=ps[1][:])
            nc.scalar.dma_start(out=out[:, 512:1024],
                                in_=out_sb[:, 512:1024])

    nc.compile()
    return nc


def shard_inputs(x, weight_base, weight_tag, tile_mask, bias,
                 mode="p2f8", ndr=16):
    """Host-side data prep: ternarize, shard, pack, re-layout."""
    import ml_dtypes
    in_features = x.shape[1]
    batch = x.shape[0]
    out_features = weight_base.shape[0]
    out_core = out_features // N_CORES
    KC = in_features // 128
    GC = KC // 4

    tern = _ternarize(np.asarray(weight_base, np.float32),
                      np.asarray(weight_tag, np.float32),
                      np.asarray(tile_mask, np.float32))

    xs = np.asarray(x, np.float32) * 0.5
    xT = xs.T.reshape(KC, 128, batch).transpose(1, 0, 2)   # [p, k, b]
    xTc = np.ascontiguousarray(
        xT.reshape(128, KC * batch).astype(ml_dtypes.bfloat16))
    x8c = np.ascontiguousarray(
        xT[:, :ndr, :].reshape(128, ndr * batch)
        .astype(ml_dtypes.float8_e4m3)) if ndr else None
    bias_bf = np.asarray(bias, np.float32).astype(ml_dtypes.bfloat16)

    in_maps = []
    for c in range(N_CORES):
        o0, o1 = c * out_core, (c + 1) * out_core
        tt = tern[o0:o1, :].T                       # [in, out_core] int8
        m = {"xTc": xTc,
             "bias": np.ascontiguousarray(bias_bf[o0:o1].reshape(1, -1))}
        if x8c is not None:
            m["x8c"] = x8c
        code = (tt & 3).astype(np.uint16)           # -1->3, 0->0, +1->1
        code = code.reshape(GC, 4, 128, out_core)
        byte = np.zeros((GC, 128, out_core), np.uint16)
        for qq in range(4):
            byte |= code[:, qq] << np.uint16(6 - 2 * qq)
        # uint16 lane jj = cols (2jj, 2jj+1), little-endian
        pk = byte[:, :, 0::2] | (byte[:, :, 1::2] << np.uint16(8))
        m["wpk"] = np.ascontiguousarray(
            pk.transpose(1, 0, 2).reshape(128, GC * (out_core // 2)))
        in_maps.append(m)
    return in_maps, mode


_GRAPH_CACHE = {}


def _get_graph(in_features, out_core, batch, **kw):
    key = (in_features, out_core, batch, tuple(sorted(kw.items())))
    if key not in _GRAPH_CACHE:
        _GRAPH_CACHE[key] = build_graph(in_features, out_core, batch, **kw)
    return _GRAPH_CACHE[key]


def run_sharded(in_maps, trace=False, mode="p2f8", **kw):
    batch = BATCH
    in_features = in_maps[0]["xTc"].shape[1] * 128 // batch
    out_core = in_maps[0]["wpk"].shape[1] * 8 // (in_features // 128)
    if "x8c" in in_maps[0]:
        kw.setdefault("ndr", in_maps[0]["x8c"].shape[1] // batch)
    else:
        kw.setdefault("ndr", 0)
    nc = _get_graph(in_features, out_core, batch, **kw)
    res = run_bass_kernel_spmd(nc, in_maps, core_ids=list(range(N_CORES)),
                               trace=trace)
    full = np.concatenate([res.results[i]["out"] for i in range(N_CORES)],
                          axis=1)
    return full, res


def kernel(x, weight_base, weight_tag, tile_mask, bias):
    x = np.ascontiguousarray(np.asarray(x, dtype=np.float32))
    weight_base = np.ascontiguousarray(np.asarray(weight_base, np.float32))
    weight_tag = np.ascontiguousarray(np.asarray(weight_tag, np.float32))
    tile_mask = np.ascontiguousarray(np.asarray(tile_mask, np.float32))
    bias = np.ascontiguousarray(np.asarray(bias, np.float32))
    in_maps, mode = shard_inputs(x, weight_base, weight_tag, tile_mask,
                                 bias)
    full, _ = run_sharded(in_maps, trace=False, mode=mode)
    return np.ascontiguousarray(full.astype(np.float32))
